# revision 1
# baseline (speedup 1.0000x reference)
"""GCN layer (GCNConv + ReLU) Bass kernel for 8 Trainium2 NeuronCores.

Reference computation (PyG GCNConv with self-loops, eval mode):
    deg  = in-degree(dst) + 1                       (self loops included)
    norm_e = deg^-1/2[src_e] * deg^-1/2[dst_e]
    out  = relu( segment_sum_dst( (x @ W)[src] * norm ) + b )
         = relu( segment_sum_dst( x[src] * norm ) @ W + b )   (W applied last)

Device strategy (per core, SPMD over 8 cores):
  - dst nodes are bin-packed into uniform "chunks" of <=256 slot positions;
    each core owns chunks_per_core chunks.  Edges are grouped by dst chunk
    and split into two streams by src range (int16 gather-index limit).
  - For each chunk: dma_gather pulls x[src] rows (512B each) from HBM into
    SBUF, a one-hot selection matrix S[e, dst_slot] = (iota==dst_e)*norm_e
    is built on VectorE, and TensorE accumulates
        aggT[cin, dst_slot] += sum_e G[e, cin] * S[e, dst_slot]
    into PSUM across all edge blocks of the chunk.
  - Finally aggT @ W is computed (W stationary), bias+ReLU applied on
    ScalarE, and the transposed [cout, dst] tile is stored; the host
    unpermutes/transposes to the full [N, C] output.
"""

import os

import numpy as np

import concourse.bacc as bacc
import concourse.bass as bass
import concourse.mybir as mybir
import concourse.tile as tile
from concourse.bass_utils import run_bass_kernel_spmd

N_CORES = 8
CHUNK_W = 256  # dst slots per chunk == PSUM tile free dim
SPLIT = 32768  # gather table split point (int16 index limit)
NODES_PER_CHUNK = 250

# knobs (env overridable for experiments)
USE_F32R = os.environ.get("GCN_F32R", "1") == "1"
GATHER_DTYPE = os.environ.get("GCN_GATHER_DTYPE", "f32")  # "f32" | "bf16"

LAST_RUN_INFO = {}


def _host_prep(x, edge_index):
    """Host-side sharding: chunk assignment, edge bucketing, index layout."""
    N, C = x.shape
    src = np.asarray(edge_index[0], dtype=np.int64)
    dst = np.asarray(edge_index[1], dtype=np.int64)
    loops = np.arange(N, dtype=np.int64)
    src = np.concatenate([src, loops])
    dst = np.concatenate([dst, loops])

    deg = np.bincount(dst, minlength=N)
    dinv = (1.0 / np.sqrt(deg.astype(np.float64))).astype(np.float32)
    norm = dinv[src] * dinv[dst]

    chunks_per_core = int(np.ceil(N / (N_CORES * NODES_PER_CHUNK)))
    nchunks = N_CORES * chunks_per_core

    # Balance chunks by in-degree: snake round-robin over degree-sorted nodes.
    order = np.argsort(-deg, kind="stable")
    r = np.arange(N)
    pos = r % nchunks
    rnd = r // nchunks
    ch = np.where(rnd % 2 == 0, pos, nchunks - 1 - pos)
    chunk_of = np.empty(N, np.int64)
    slot_of = np.empty(N, np.int64)
    chunk_of[order] = ch
    slot_of[order] = rnd
    assert slot_of.max() < CHUNK_W

    e_chunk = chunk_of[dst]
    e_slot = slot_of[dst]
    islo = src < SPLIT
    key = e_chunk * 2 + (~islo).astype(np.int64)
    perm = np.argsort(key, kind="stable")
    ks = key[perm]
    gsz = np.bincount(key, minlength=2 * nchunks)
    gstart = np.zeros(2 * nchunks, np.int64)
    gstart[1:] = np.cumsum(gsz)[:-1]
    rank = np.arange(len(ks)) - gstart[ks]

    cnt_lo = gsz[0::2]
    cnt_hi = gsz[1::2]
    B_lo = max(1, int(np.ceil(cnt_lo.max() / 128)))
    B_hi = max(1, int(np.ceil(cnt_hi.max() / 128)))
    S_lo, S_hi = B_lo * 128, B_hi * 128
    S_tot = S_lo + S_hi

    flat_idx = np.zeros(nchunks * S_tot, np.int64)
    flat_dst = np.zeros(nchunks * S_tot, np.float32)
    flat_nrm = np.zeros(nchunks * S_tot, np.float32)
    cc = ks // 2
    s = ks % 2
    col = cc * S_tot + np.where(s == 0, rank, S_lo + rank)
    ss = src[perm]
    flat_idx[col] = np.where(s == 0, ss, ss - SPLIT)
    flat_dst[col] = e_slot[perm].astype(np.float32)
    flat_nrm[col] = norm[perm]
    assert flat_idx.max() < SPLIT and flat_idx.min() >= 0
    flat_idx = flat_idx.astype(np.int16)

    A_idx = flat_idx.reshape(nchunks, S_tot)
    ilo = A_idx[:, :S_lo].reshape(nchunks, B_lo * 8, 16)
    ihi = A_idx[:, S_lo:].reshape(nchunks, B_hi * 8, 16)
    A_dst = flat_dst.reshape(nchunks, S_tot // 128, 128)
    A_nrm = flat_nrm.reshape(nchunks, S_tot // 128, 128)

    per_core = []
    cpc = chunks_per_core
    for k in range(N_CORES):
        sl = slice(k * cpc, (k + 1) * cpc)
        # gather idx layout: logical idx i -> [i%16 (replicated x8), i//16]
        vlo = ilo[sl].transpose(2, 0, 1).reshape(16, -1)
        vhi = ihi[sl].transpose(2, 0, 1).reshape(16, -1)
        per_core.append(
            dict(
                idx_lo=np.ascontiguousarray(np.tile(vlo, (8, 1))),
                idx_hi=np.ascontiguousarray(np.tile(vhi, (8, 1))),
                dstslot=np.ascontiguousarray(
                    A_dst[sl].transpose(2, 0, 1).reshape(128, -1)
                ),
                normv=np.ascontiguousarray(
                    A_nrm[sl].transpose(2, 0, 1).reshape(128, -1)
                ),
            )
        )

    meta = dict(
        N=N,
        C=C,
        B_lo=B_lo,
        B_hi=B_hi,
        cpc=cpc,
        nchunks=nchunks,
        chunk_of=chunk_of,
        slot_of=slot_of,
    )
    return per_core, meta


def _pieces(nblocks, max_blocks):
    """Split nblocks into near-even contiguous pieces of <= max_blocks."""
    npieces = -(-nblocks // max_blocks)
    out = []
    a = 0
    for p in range(npieces):
        b = a + (nblocks - a) // (npieces - p) + (1 if (nblocks - a) % (npieces - p) else 0)
        out.append((a, b))
        a = b
    return out


def _build_program(N, C, B_lo, B_hi, cpc):
    f32 = mybir.dt.float32
    bf16 = mybir.dt.bfloat16
    i16 = mybir.dt.int16
    if GATHER_DTYPE == "bf16":
        gdt = bf16
    elif USE_F32R:
        gdt = mybir.dt.float32r
    else:
        gdt = f32
    B = B_lo + B_hi
    S_lo, S_hi = B_lo * 128, B_hi * 128

    nc = bacc.Bacc(None, target_bir_lowering=False, debug=False)

    x_d = nc.dram_tensor("xg", [N, C], gdt, kind="ExternalInput")
    ilo_d = nc.dram_tensor("idx_lo", [128, cpc * B_lo * 8], i16, kind="ExternalInput")
    ihi_d = nc.dram_tensor("idx_hi", [128, cpc * B_hi * 8], i16, kind="ExternalInput")
    dst_d = nc.dram_tensor("dstslot", [128, cpc * B], f32, kind="ExternalInput")
    nrm_d = nc.dram_tensor("normv", [128, cpc * B], f32, kind="ExternalInput")
    iota_dt = bf16 if gdt == bf16 else f32
    iota_d = nc.dram_tensor("iota", [128, CHUNK_W], iota_dt, kind="ExternalInput")
    w_d = nc.dram_tensor("weight", [C, C], f32, kind="ExternalInput")
    b_d = nc.dram_tensor("bias", [128, 1], f32, kind="ExternalInput")
    out_d = nc.dram_tensor("out", [128, cpc * CHUNK_W], f32, kind="ExternalOutput")

    with tile.TileContext(nc) as tc:
        with (
            tc.tile_pool(name="const", bufs=1) as constp,
            tc.tile_pool(name="gat_lo", bufs=3) as glop,
            tc.tile_pool(name="gat_hi", bufs=3) as ghip,
            tc.tile_pool(name="idxp", bufs=3) as idxp,
            tc.tile_pool(name="sel", bufs=6) as selp,
            tc.tile_pool(name="aggs", bufs=3) as aggsp,
            tc.tile_pool(name="outs", bufs=3) as outsp,
            tc.tile_pool(name="pagg", bufs=3, space="PSUM") as pagg,
            tc.tile_pool(name="pout", bufs=2, space="PSUM") as pout,
        ):
            iota_t = constp.tile([128, CHUNK_W], iota_dt, tag="iota")
            nc.sync.dma_start(iota_t[:], iota_d[:])
            w_t = constp.tile([C, C], f32, tag="w")
            nc.sync.dma_start(w_t[:], w_d[:])
            bias_t = constp.tile([128, 1], f32, tag="bias")
            nc.sync.dma_start(bias_t[:], b_d[:])
            dst_t = constp.tile([128, cpc * B], f32, tag="dst")
            nc.sync.dma_start(dst_t[:], dst_d[:])
            nrm_t = constp.tile([128, cpc * B], f32, tag="nrm")
            nc.sync.dma_start(nrm_t[:], nrm_d[:])

            x_lo = x_d[0:SPLIT, :]
            x_hi = x_d[SPLIT:N, :]

            for c in range(cpc):
                ilo_t = idxp.tile([128, B_lo * 8], i16, tag="ilo")
                nc.sync.dma_start(
                    ilo_t[:], ilo_d[:, c * B_lo * 8 : (c + 1) * B_lo * 8]
                )
                ihi_t = idxp.tile([128, B_hi * 8], i16, tag="ihi")
                nc.sync.dma_start(
                    ihi_t[:], ihi_d[:, c * B_hi * 8 : (c + 1) * B_hi * 8]
                )
                # dma_gather with single_packet caps at 64 descs/engine = 1024
                # idxs per instruction (HW-verified: 1280 wedges the device).
                glo_t = glop.tile([128, B_lo, C], gdt, tag="glo")
                for kb0, kb1 in _pieces(B_lo, 8):
                    n = (kb1 - kb0) * 128
                    nc.gpsimd.dma_gather(
                        glo_t[:, kb0:kb1, :],
                        x_lo,
                        ilo_t[:, kb0 * 8 : kb1 * 8],
                        n,
                        n,
                        C,
                    )
                ghi_t = ghip.tile([128, B_hi, C], gdt, tag="ghi")
                for kb0, kb1 in _pieces(B_hi, 8):
                    n = (kb1 - kb0) * 128
                    nc.gpsimd.dma_gather(
                        ghi_t[:, kb0:kb1, :],
                        x_hi,
                        ihi_t[:, kb0 * 8 : kb1 * 8],
                        n,
                        n,
                        C,
                    )

                agg_t = pagg.tile([128, CHUNK_W], mybir.dt.float32, tag="agg")
                for b in range(B):
                    gb = c * B + b
                    sel_t = selp.tile([128, CHUNK_W], gdt, tag="sel")
                    nc.vector.tensor_scalar(
                        sel_t[:],
                        iota_t[:],
                        dst_t[:, gb : gb + 1],
                        nrm_t[:, gb : gb + 1],
                        mybir.AluOpType.is_equal,
                        mybir.AluOpType.mult,
                    )
                    g_ap = glo_t[:, b, :] if b < B_lo else ghi_t[:, b - B_lo, :]
                    s_ap = sel_t[:]
                    nc.tensor.matmul(
                        agg_t[:],
                        lhsT=g_ap,
                        rhs=s_ap,
                        start=(b == 0),
                        stop=(b == B - 1),
                    )
                aggs_t = aggsp.tile([128, CHUNK_W], f32, tag="aggs")
                nc.scalar.copy(aggs_t[:], agg_t[:])
                outp_t = pout.tile([128, CHUNK_W], mybir.dt.float32, tag="outp")
                nc.tensor.matmul(
                    outp_t[:], lhsT=w_t[:], rhs=aggs_t[:], start=True, stop=True
                )
                outs_t = outsp.tile([128, CHUNK_W], f32, tag="outs")
                nc.scalar.activation(
                    outs_t[:],
                    outp_t[:],
                    mybir.ActivationFunctionType.Relu,
                    bias=bias_t[:, 0:1],
                    scale=1.0,
                )
                nc.sync.dma_start(out_d[:, c * CHUNK_W : (c + 1) * CHUNK_W], outs_t[:])
    nc.compile()
    return nc


def _make_in_maps(x, weight, bias, per_core, meta):
    C = meta["C"]
    gnp = np.float32
    xg = np.ascontiguousarray(x.astype(np.float32))
    if GATHER_DTYPE == "bf16":
        import ml_dtypes

        gnp = ml_dtypes.bfloat16
        xg = xg.astype(gnp)
    iota = np.tile(np.arange(CHUNK_W, dtype=np.float32), (128, 1)).astype(gnp)
    w = np.ascontiguousarray(weight.astype(np.float32))
    bvec = np.zeros((128, 1), np.float32)
    bvec[: len(bias), 0] = bias.astype(np.float32)
    in_maps = []
    for k in range(N_CORES):
        pc = per_core[k]
        in_maps.append(
            dict(
                xg=xg,
                idx_lo=pc["idx_lo"],
                idx_hi=pc["idx_hi"],
                dstslot=pc["dstslot"],
                normv=pc["normv"],
                iota=iota,
                weight=w,
                bias=bvec,
            )
        )
    return in_maps


def _unshard(results, meta):
    outs = [np.asarray(results[k]["out"], dtype=np.float32) for k in range(N_CORES)]
    big = np.concatenate(outs, axis=1).reshape(128, meta["nchunks"], CHUNK_W)
    return np.ascontiguousarray(big[:, meta["chunk_of"], meta["slot_of"]].T)


def kernel(x, edge_index, weight, bias):
    x = np.asarray(x)
    per_core, meta = _host_prep(x, edge_index)
    nc = _build_program(meta["N"], meta["C"], meta["B_lo"], meta["B_hi"], meta["cpc"])
    in_maps = _make_in_maps(x, np.asarray(weight), np.asarray(bias), per_core, meta)
    res = run_bass_kernel_spmd(
        nc,
        in_maps,
        list(range(N_CORES)),
        trace=os.environ.get("GCN_TRACE", "0") == "1",
    )
    LAST_RUN_INFO["exec_time_ns"] = res.exec_time_ns
    LAST_RUN_INFO["meta"] = {k: v for k, v in meta.items() if np.isscalar(v)}
    return _unshard(res.results, meta)



# revision 6
# speedup vs baseline: 1.8709x; 1.8709x over previous
"""GCN layer (GCNConv + ReLU) Bass kernel for 8 Trainium2 NeuronCores.

Reference computation (PyG GCNConv with self-loops, eval mode):
    deg  = in-degree(dst) + 1                       (self loops included)
    norm_e = deg^-1/2[src_e] * deg^-1/2[dst_e]
    out  = relu( segment_sum_dst( (x @ W)[src] * norm ) + b )
         = relu( segment_sum_dst( x[src] * norm ) @ W + b )   (W applied last)

Device strategy (per core, SPMD over 8 cores):
  - dst nodes are bin-packed into uniform "chunks" of <=256 slot positions;
    each core owns chunks_per_core chunks.  Edges are grouped by dst chunk
    and split into two streams by src range (int16 gather-index limit).
  - For each chunk: dma_gather pulls x[src] rows (512B each) from HBM into
    SBUF, a one-hot selection matrix S[e, dst_slot] = (iota==dst_e)*norm_e
    is built on VectorE, and TensorE accumulates
        aggT[cin, dst_slot] += sum_e G[e, cin] * S[e, dst_slot]
    into PSUM across all edge blocks of the chunk.
  - Finally aggT @ W is computed (W stationary), bias+ReLU applied on
    ScalarE, and the transposed [cout, dst] tile is stored; the host
    unpermutes/transposes to the full [N, C] output.
"""

import os

import numpy as np

import concourse.bacc as bacc
import concourse.bass as bass
import concourse.mybir as mybir
import concourse.tile as tile
from concourse.bass_utils import run_bass_kernel_spmd

N_CORES = 8
CHUNK_W = 256  # dst slots per chunk == PSUM tile free dim
SPLIT = 32768  # gather table split point (int16 index limit)
NODES_PER_CHUNK = 250

# knobs (env overridable for experiments)
USE_F32R = os.environ.get("GCN_F32R", "1") == "1"
GATHER_DTYPE = os.environ.get("GCN_GATHER_DTYPE", "bf16")  # "f32" | "bf16"
N_QUEUES = int(os.environ.get("GCN_QUEUES", "4"))  # SWDGE queues (1..4)

LAST_RUN_INFO = {}


def _host_prep(x, edge_index):
    """Host-side sharding: chunk assignment, edge bucketing, index layout."""
    N, C = x.shape
    src = np.asarray(edge_index[0], dtype=np.int64)
    dst = np.asarray(edge_index[1], dtype=np.int64)
    loops = np.arange(N, dtype=np.int64)
    src = np.concatenate([src, loops])
    dst = np.concatenate([dst, loops])

    deg = np.bincount(dst, minlength=N)
    dinv = (1.0 / np.sqrt(deg.astype(np.float64))).astype(np.float32)
    norm = dinv[src] * dinv[dst]

    chunks_per_core = int(np.ceil(N / (N_CORES * NODES_PER_CHUNK)))
    nchunks = N_CORES * chunks_per_core

    # Balance chunks by in-degree: snake round-robin over degree-sorted nodes.
    order = np.argsort(-deg, kind="stable")
    r = np.arange(N)
    pos = r % nchunks
    rnd = r // nchunks
    ch = np.where(rnd % 2 == 0, pos, nchunks - 1 - pos)
    chunk_of = np.empty(N, np.int64)
    slot_of = np.empty(N, np.int64)
    chunk_of[order] = ch
    slot_of[order] = rnd
    assert slot_of.max() < CHUNK_W

    e_chunk = chunk_of[dst]
    e_slot = slot_of[dst]
    islo = src < SPLIT
    key = e_chunk * 2 + (~islo).astype(np.int64)
    perm = np.argsort(key, kind="stable")
    ks = key[perm]
    gsz = np.bincount(key, minlength=2 * nchunks)
    gstart = np.zeros(2 * nchunks, np.int64)
    gstart[1:] = np.cumsum(gsz)[:-1]
    rank = np.arange(len(ks)) - gstart[ks]

    cnt_lo = gsz[0::2]
    cnt_hi = gsz[1::2]
    B_lo = max(1, int(np.ceil(cnt_lo.max() / 128)))
    B_hi = max(1, int(np.ceil(cnt_hi.max() / 128)))
    S_lo, S_hi = B_lo * 128, B_hi * 128
    S_tot = S_lo + S_hi

    flat_idx = np.zeros(nchunks * S_tot, np.int64)
    flat_dst = np.zeros(nchunks * S_tot, np.float32)
    flat_nrm = np.zeros(nchunks * S_tot, np.float32)
    cc = ks // 2
    s = ks % 2
    col = cc * S_tot + np.where(s == 0, rank, S_lo + rank)
    ss = src[perm]
    flat_idx[col] = np.where(s == 0, ss, ss - SPLIT)
    flat_dst[col] = e_slot[perm].astype(np.float32)
    flat_nrm[col] = norm[perm]
    assert flat_idx.max() < SPLIT and flat_idx.min() >= 0
    flat_idx = flat_idx.astype(np.int16)

    A_idx = flat_idx.reshape(nchunks, S_tot)
    ilo = A_idx[:, :S_lo].reshape(nchunks, B_lo * 8, 16)
    ihi = A_idx[:, S_lo:].reshape(nchunks, B_hi * 8, 16)
    A_dst = flat_dst.reshape(nchunks, S_tot // 128, 128)
    A_nrm = flat_nrm.reshape(nchunks, S_tot // 128, 128)

    per_core = []
    cpc = chunks_per_core
    for k in range(N_CORES):
        sl = slice(k * cpc, (k + 1) * cpc)
        # gather idx layout: logical idx i -> [i%16 (replicated x8), i//16]
        vlo = ilo[sl].transpose(2, 0, 1).reshape(16, -1)
        vhi = ihi[sl].transpose(2, 0, 1).reshape(16, -1)
        per_core.append(
            dict(
                idx_lo=np.ascontiguousarray(np.tile(vlo, (8, 1))),
                idx_hi=np.ascontiguousarray(np.tile(vhi, (8, 1))),
                dstslot=np.ascontiguousarray(
                    A_dst[sl].transpose(2, 0, 1).reshape(128, -1)
                ),
                normv=np.ascontiguousarray(
                    A_nrm[sl].transpose(2, 0, 1).reshape(128, -1)
                ),
            )
        )

    meta = dict(
        N=N,
        C=C,
        B_lo=B_lo,
        B_hi=B_hi,
        cpc=cpc,
        nchunks=nchunks,
        chunk_of=chunk_of,
        slot_of=slot_of,
    )
    return per_core, meta


def _pieces(nblocks, max_blocks):
    """Split nblocks into near-even contiguous pieces of <= max_blocks."""
    npieces = -(-nblocks // max_blocks)
    out = []
    a = 0
    for p in range(npieces):
        b = a + (nblocks - a) // (npieces - p) + (1 if (nblocks - a) % (npieces - p) else 0)
        out.append((a, b))
        a = b
    return out


def _build_program(N, C, B_lo, B_hi, cpc):
    f32 = mybir.dt.float32
    bf16 = mybir.dt.bfloat16
    i16 = mybir.dt.int16
    if GATHER_DTYPE == "bf16":
        gdt = bf16
    elif USE_F32R:
        gdt = mybir.dt.float32r
    else:
        gdt = f32
    B = B_lo + B_hi
    S_lo, S_hi = B_lo * 128, B_hi * 128

    nc = bacc.Bacc(
        None, target_bir_lowering=False, debug=False, num_swdge_queues=N_QUEUES
    )

    # dst/norm tables in the sel-build dtype so the one-hot tensor_scalar is
    # all-16-bit (DVE 2x/4x perf mode) when gathering bf16.
    sel_dt = bf16 if gdt == bf16 else f32
    x_d = nc.dram_tensor("xg", [N, C], gdt, kind="ExternalInput")
    ilo_d = nc.dram_tensor("idx_lo", [128, cpc * B_lo * 8], i16, kind="ExternalInput")
    ihi_d = nc.dram_tensor("idx_hi", [128, cpc * B_hi * 8], i16, kind="ExternalInput")
    dst_d = nc.dram_tensor("dstslot", [128, cpc * B], f32, kind="ExternalInput")
    nrm_d = nc.dram_tensor("normv", [128, cpc * B], f32, kind="ExternalInput")
    iota_dt = sel_dt
    iota_d = nc.dram_tensor("iota", [128, CHUNK_W], iota_dt, kind="ExternalInput")
    w_d = nc.dram_tensor("weight", [C, C], f32, kind="ExternalInput")
    b_d = nc.dram_tensor("bias", [128, 1], f32, kind="ExternalInput")
    out_d = nc.dram_tensor("out", [128, cpc * CHUNK_W], f32, kind="ExternalOutput")

    with tile.TileContext(nc) as tc:
        with (
            tc.tile_pool(name="const", bufs=1) as constp,
            tc.tile_pool(name="gat_lo", bufs=3) as glop,
            tc.tile_pool(name="gat_hi", bufs=3) as ghip,
            tc.tile_pool(name="idxp", bufs=3) as idxp,
            tc.tile_pool(name="sel", bufs=6) as selp,
            tc.tile_pool(name="aggs", bufs=3) as aggsp,
            tc.tile_pool(name="outs", bufs=3) as outsp,
            tc.tile_pool(name="pagg", bufs=3, space="PSUM") as pagg,
            tc.tile_pool(name="pout", bufs=2, space="PSUM") as pout,
        ):
            iota_t = constp.tile([128, CHUNK_W], iota_dt, tag="iota")
            nc.sync.dma_start(iota_t[:], iota_d[:])
            w_t = constp.tile([C, C], f32, tag="w")
            nc.sync.dma_start(w_t[:], w_d[:])
            bias_t = constp.tile([128, 1], f32, tag="bias")
            nc.sync.dma_start(bias_t[:], b_d[:])
            dst_t = constp.tile([128, cpc * B], f32, tag="dst")
            nc.sync.dma_start(dst_t[:], dst_d[:])
            nrm_t = constp.tile([128, cpc * B], f32, tag="nrm")
            nc.sync.dma_start(nrm_t[:], nrm_d[:])

            x_lo = x_d[0:SPLIT, :]
            x_hi = x_d[SPLIT:N, :]

            qc = 0  # rotate dma_gather instructions across SWDGE queues
            for c in range(cpc):
                ilo_t = idxp.tile([128, B_lo * 8], i16, tag="ilo")
                nc.sync.dma_start(
                    ilo_t[:], ilo_d[:, c * B_lo * 8 : (c + 1) * B_lo * 8]
                )
                ihi_t = idxp.tile([128, B_hi * 8], i16, tag="ihi")
                nc.sync.dma_start(
                    ihi_t[:], ihi_d[:, c * B_hi * 8 : (c + 1) * B_hi * 8]
                )
                # dma_gather with single_packet caps at 64 descs/engine = 1024
                # idxs per instruction (HW-verified: 1280 wedges the device).
                glo_t = glop.tile([128, B_lo, C], gdt, tag="glo")
                for kb0, kb1 in _pieces(B_lo, 8):
                    n = (kb1 - kb0) * 128
                    nc.gpsimd.dma_gather(
                        glo_t[:, kb0:kb1, :],
                        x_lo,
                        ilo_t[:, kb0 * 8 : kb1 * 8],
                        n,
                        n,
                        C,
                        queue_num=qc % N_QUEUES,
                    )
                    qc += 1
                ghi_t = ghip.tile([128, B_hi, C], gdt, tag="ghi")
                for kb0, kb1 in _pieces(B_hi, 8):
                    n = (kb1 - kb0) * 128
                    nc.gpsimd.dma_gather(
                        ghi_t[:, kb0:kb1, :],
                        x_hi,
                        ihi_t[:, kb0 * 8 : kb1 * 8],
                        n,
                        n,
                        C,
                        queue_num=qc % N_QUEUES,
                    )
                    qc += 1

                agg_t = pagg.tile([128, CHUNK_W], mybir.dt.float32, tag="agg")
                for b in range(B):
                    gb = c * B + b
                    sel_t = selp.tile([128, CHUNK_W], gdt, tag="sel")
                    nc.vector.tensor_scalar(
                        sel_t[:],
                        iota_t[:],
                        dst_t[:, gb : gb + 1],
                        nrm_t[:, gb : gb + 1],
                        mybir.AluOpType.is_equal,
                        mybir.AluOpType.mult,
                    )
                    g_ap = glo_t[:, b, :] if b < B_lo else ghi_t[:, b - B_lo, :]
                    s_ap = sel_t[:]
                    nc.tensor.matmul(
                        agg_t[:],
                        lhsT=g_ap,
                        rhs=s_ap,
                        start=(b == 0),
                        stop=(b == B - 1),
                    )
                aggs_t = aggsp.tile([128, CHUNK_W], f32, tag="aggs")
                nc.scalar.copy(aggs_t[:], agg_t[:])
                outp_t = pout.tile([128, CHUNK_W], mybir.dt.float32, tag="outp")
                nc.tensor.matmul(
                    outp_t[:], lhsT=w_t[:], rhs=aggs_t[:], start=True, stop=True
                )
                outs_t = outsp.tile([128, CHUNK_W], f32, tag="outs")
                nc.scalar.activation(
                    outs_t[:],
                    outp_t[:],
                    mybir.ActivationFunctionType.Relu,
                    bias=bias_t[:, 0:1],
                    scale=1.0,
                )
                nc.sync.dma_start(out_d[:, c * CHUNK_W : (c + 1) * CHUNK_W], outs_t[:])
    nc.compile()
    return nc


def _make_in_maps(x, weight, bias, per_core, meta):
    C = meta["C"]
    gnp = np.float32
    selnp = np.float32
    xg = np.ascontiguousarray(x.astype(np.float32))
    if GATHER_DTYPE == "bf16":
        import ml_dtypes

        gnp = ml_dtypes.bfloat16
        selnp = ml_dtypes.bfloat16
        xg = xg.astype(gnp)
    iota = np.tile(np.arange(CHUNK_W, dtype=np.float32), (128, 1)).astype(selnp)
    w = np.ascontiguousarray(weight.astype(np.float32))
    bvec = np.zeros((128, 1), np.float32)
    bvec[: len(bias), 0] = bias.astype(np.float32)
    in_maps = []
    for k in range(N_CORES):
        pc = per_core[k]
        in_maps.append(
            dict(
                xg=xg,
                idx_lo=pc["idx_lo"],
                idx_hi=pc["idx_hi"],
                dstslot=pc["dstslot"],
                normv=pc["normv"],
                iota=iota,
                weight=w,
                bias=bvec,
            )
        )
    return in_maps


def _unshard(results, meta):
    outs = [np.asarray(results[k]["out"], dtype=np.float32) for k in range(N_CORES)]
    big = np.concatenate(outs, axis=1).reshape(128, meta["nchunks"], CHUNK_W)
    return np.ascontiguousarray(big[:, meta["chunk_of"], meta["slot_of"]].T)


def kernel(x, edge_index, weight, bias):
    x = np.asarray(x)
    per_core, meta = _host_prep(x, edge_index)
    nc = _build_program(meta["N"], meta["C"], meta["B_lo"], meta["B_hi"], meta["cpc"])
    in_maps = _make_in_maps(x, np.asarray(weight), np.asarray(bias), per_core, meta)
    res = run_bass_kernel_spmd(
        nc,
        in_maps,
        list(range(N_CORES)),
        trace=os.environ.get("GCN_TRACE", "0") == "1",
    )
    LAST_RUN_INFO["exec_time_ns"] = res.exec_time_ns
    LAST_RUN_INFO["meta"] = {k: v for k, v in meta.items() if np.isscalar(v)}
    return _unshard(res.results, meta)



# revision 7
# speedup vs baseline: 2.1561x; 1.1525x over previous
"""GCN layer (GCNConv + ReLU) Bass kernel for 8 Trainium2 NeuronCores.

Reference computation (PyG GCNConv with self-loops, eval mode):
    deg  = in-degree(dst) + 1                       (self loops included)
    norm_e = deg^-1/2[src_e] * deg^-1/2[dst_e]
    out  = relu( segment_sum_dst( (x @ W)[src] * norm ) + b )
         = relu( segment_sum_dst( x[src] * norm ) @ W + b )   (W applied last)

Device strategy (per core, SPMD over 8 cores):
  - dst nodes are bin-packed into uniform "chunks" of <=256 slot positions;
    each core owns cpc chunks.  Edges are grouped by dst chunk and split
    into two streams by src range (int16 gather-index limit).
  - Each core's chunks are rank-ordered by edge count (descending) so the
    c-th chunk has a similar block count on every core; per-rank loop
    bounds are the max over cores, keeping the SPMD program shared while
    nearly eliminating pad gathers.
  - For each chunk: dma_gather (bf16 rows, 4 SWDGE queues round-robin)
    pulls x[src] rows from HBM into SBUF, a one-hot selection matrix
    S[e, dst_slot] = (iota==dst_e)*norm_e is built on VectorE (or on
    ScalarE via abs/relu, to offload the DVE<->GpSimd shared SBUF port),
    and TensorE accumulates
        aggT[cin, dst_slot] += sum_e G[e, cin] * S[e, dst_slot]
    into PSUM across all edge blocks of the chunk.
  - Finally aggT @ W is computed (W stationary), bias+ReLU applied on
    ScalarE, and the transposed [cout, dst] tile is stored; the host
    unpermutes/transposes to the full [N, C] output.
"""

import os

import numpy as np

import concourse.bacc as bacc
import concourse.bass as bass
import concourse.mybir as mybir
import concourse.tile as tile
from concourse.bass_utils import run_bass_kernel_spmd

N_CORES = 8
CHUNK_W = 256  # dst slots per chunk == PSUM tile free dim
SPLIT = 32768  # gather table split point (int16 index limit)
NODES_PER_CHUNK = 250

# knobs (env overridable for experiments)
N_QUEUES = int(os.environ.get("GCN_QUEUES", "4"))  # SWDGE queues (1..4)
ACT_MOD = int(os.environ.get("GCN_ACT_MOD", "3"))  # every ACT_MOD-th sel on ScalarE
GATHER_DTYPE = os.environ.get("GCN_GATHER_DTYPE", "bf16")

LAST_RUN_INFO = {}


def _host_prep(x, edge_index):
    """Host-side sharding: chunk assignment, edge bucketing, index layout."""
    N, C = x.shape
    src = np.asarray(edge_index[0], dtype=np.int64)
    dst = np.asarray(edge_index[1], dtype=np.int64)
    loops = np.arange(N, dtype=np.int64)
    src = np.concatenate([src, loops])
    dst = np.concatenate([dst, loops])

    deg = np.bincount(dst, minlength=N)
    dinv = (1.0 / np.sqrt(deg.astype(np.float64))).astype(np.float32)
    norm = dinv[src] * dinv[dst]

    cpc = int(np.ceil(N / (N_CORES * NODES_PER_CHUNK)))
    nchunks = N_CORES * cpc

    # Balance chunks by in-degree: snake round-robin over degree-sorted nodes.
    order = np.argsort(-deg, kind="stable")
    r = np.arange(N)
    pos = r % nchunks
    rnd = r // nchunks
    ch = np.where(rnd % 2 == 0, pos, nchunks - 1 - pos)
    chunk_of = np.empty(N, np.int64)
    slot_of = np.empty(N, np.int64)
    chunk_of[order] = ch
    slot_of[order] = rnd
    assert slot_of.max() < CHUNK_W

    e_chunk = chunk_of[dst]
    e_slot = slot_of[dst]
    islo = src < SPLIT
    cnt_lo = np.bincount(e_chunk[islo], minlength=nchunks)
    cnt_hi = np.bincount(e_chunk[~islo], minlength=nchunks)

    # Rank-order each core's chunks by total count (desc) and take per-rank
    # maxima across cores so all cores share one set of loop bounds.
    rank_of_chunk = np.empty(nchunks, np.int64)
    chunk_at = np.empty((N_CORES, cpc), np.int64)
    for k in range(N_CORES):
        mine = np.arange(k * cpc, (k + 1) * cpc)
        o = np.argsort(-(cnt_lo[mine] + cnt_hi[mine]), kind="stable")
        chunk_at[k] = mine[o]
        rank_of_chunk[mine[o]] = np.arange(cpc)

    B_lo = [
        int(np.ceil(max(cnt_lo[chunk_at[k][c]] for k in range(N_CORES)) / 128))
        for c in range(cpc)
    ]
    B_hi = [
        int(np.ceil(max(cnt_hi[chunk_at[k][c]] for k in range(N_CORES)) / 128))
        for c in range(cpc)
    ]
    B_lo = [max(b, 1) for b in B_lo]
    B_hi = [max(b, 1) for b in B_hi]
    B_tot = sum(B_lo) + sum(B_hi)
    # block offset of each rank's lo / hi region in the flat layout
    off_lo, off_hi = [], []
    acc = 0
    for c in range(cpc):
        off_lo.append(acc)
        acc += B_lo[c]
        off_hi.append(acc)
        acc += B_hi[c]
    assert acc == B_tot

    # per-edge destination position in the flat per-core layout
    e_core = e_chunk // cpc
    e_rank = rank_of_chunk[e_chunk]
    off_lo_a = np.array(off_lo)
    off_hi_a = np.array(off_hi)
    # rank within the (chunk, stream) group
    key = e_chunk * 2 + (~islo).astype(np.int64)
    perm = np.argsort(key, kind="stable")
    ks = key[perm]
    gsz = np.bincount(key, minlength=2 * nchunks)
    gstart = np.zeros(2 * nchunks, np.int64)
    gstart[1:] = np.cumsum(gsz)[:-1]
    rank_in_g = np.arange(len(ks)) - gstart[ks]

    sp = src[perm]
    ep_slot = e_slot[perm]
    ep_nrm = norm[perm]
    ep_core = e_core[perm]
    ep_rank = e_rank[perm]
    ep_islo = ks % 2 == 0
    blk_base = np.where(ep_islo, off_lo_a[ep_rank], off_hi_a[ep_rank])
    pos = blk_base * 128 + rank_in_g  # position within the core's flat layout

    S = B_tot * 128
    flat_idx = np.zeros((N_CORES, S), np.int64)
    flat_dst = np.zeros((N_CORES, S), np.float32)
    flat_nrm = np.zeros((N_CORES, S), np.float32)
    flat_idx[ep_core, pos] = np.where(ep_islo, sp, sp - SPLIT)
    flat_dst[ep_core, pos] = ep_slot.astype(np.float32)
    flat_nrm[ep_core, pos] = ep_nrm
    assert flat_idx.max() < SPLIT and flat_idx.min() >= 0
    flat_idx16 = flat_idx.astype(np.int16)

    per_core = []
    for k in range(N_CORES):
        # gather idx layout: logical idx i -> [i%16 (replicated x8), i//16]
        v = flat_idx16[k].reshape(B_tot * 8, 16).T
        per_core.append(
            dict(
                idx=np.ascontiguousarray(np.tile(v, (8, 1))),
                dstslot=np.ascontiguousarray(flat_dst[k].reshape(B_tot, 128).T),
                normv=np.ascontiguousarray(flat_nrm[k].reshape(B_tot, 128).T),
            )
        )

    meta = dict(
        N=N,
        C=C,
        cpc=cpc,
        nchunks=nchunks,
        B_tot=B_tot,
        B_lo=B_lo,
        B_hi=B_hi,
        off_lo=off_lo,
        off_hi=off_hi,
        chunk_of=chunk_of,
        slot_of=slot_of,
        rank_of_chunk=rank_of_chunk,
    )
    return per_core, meta


def _pieces(nblocks, max_blocks):
    """Split nblocks into near-even contiguous pieces of <= max_blocks."""
    npieces = -(-nblocks // max_blocks)
    out = []
    a = 0
    for p in range(npieces):
        b = a + (nblocks - a) // (npieces - p) + (1 if (nblocks - a) % (npieces - p) else 0)
        out.append((a, b))
        a = b
    return out


def _build_program(meta):
    f32 = mybir.dt.float32
    bf16 = mybir.dt.bfloat16
    i16 = mybir.dt.int16
    gdt = bf16 if GATHER_DTYPE == "bf16" else f32
    N, C, cpc = meta["N"], meta["C"], meta["cpc"]
    B_lo, B_hi = meta["B_lo"], meta["B_hi"]
    off_lo, off_hi = meta["off_lo"], meta["off_hi"]
    B_tot = meta["B_tot"]
    BLmax, BHmax = max(B_lo), max(B_hi)

    nc = bacc.Bacc(
        None, target_bir_lowering=False, debug=False, num_swdge_queues=N_QUEUES
    )

    x_d = nc.dram_tensor("xg", [N, C], gdt, kind="ExternalInput")
    idx_d = nc.dram_tensor("idx", [128, B_tot * 8], i16, kind="ExternalInput")
    dst_d = nc.dram_tensor("dstslot", [128, B_tot], f32, kind="ExternalInput")
    nrm_d = nc.dram_tensor("normv", [128, B_tot], f32, kind="ExternalInput")
    ndst_d = nc.dram_tensor("ndstslot", [128, B_tot], f32, kind="ExternalInput")
    nnrm_d = nc.dram_tensor("nnormv", [128, B_tot], f32, kind="ExternalInput")
    iota_d = nc.dram_tensor("iota", [128, CHUNK_W], bf16, kind="ExternalInput")
    w_d = nc.dram_tensor("weight", [C, C], f32, kind="ExternalInput")
    b_d = nc.dram_tensor("bias", [128, 1], f32, kind="ExternalInput")
    out_d = nc.dram_tensor("out", [128, cpc * CHUNK_W], f32, kind="ExternalOutput")

    ie = mybir.AluOpType.is_equal
    mu = mybir.AluOpType.mult
    AB = mybir.ActivationFunctionType.Abs
    RL = mybir.ActivationFunctionType.Relu

    with tile.TileContext(nc) as tc:
        with (
            tc.tile_pool(name="const", bufs=1) as constp,
            tc.tile_pool(name="gat_lo", bufs=3) as glop,
            tc.tile_pool(name="gat_hi", bufs=3) as ghip,
            tc.tile_pool(name="sel", bufs=8) as selp,
            tc.tile_pool(name="tmp", bufs=4) as tmpp,
            tc.tile_pool(name="aggs", bufs=3) as aggsp,
            tc.tile_pool(name="outs", bufs=3) as outsp,
            tc.tile_pool(name="pagg", bufs=3, space="PSUM") as pagg,
            tc.tile_pool(name="pout", bufs=2, space="PSUM") as pout,
        ):
            iota_t = constp.tile([128, CHUNK_W], bf16, tag="iota")
            nc.sync.dma_start(iota_t[:], iota_d[:])
            w_t = constp.tile([C, C], f32, tag="w")
            nc.sync.dma_start(w_t[:], w_d[:])
            bias_t = constp.tile([128, 1], f32, tag="bias")
            nc.sync.dma_start(bias_t[:], b_d[:])
            idx_t = constp.tile([128, B_tot * 8], i16, tag="idx")
            nc.sync.dma_start(idx_t[:], idx_d[:])
            dst_t = constp.tile([128, B_tot], f32, tag="dst")
            nc.sync.dma_start(dst_t[:], dst_d[:])
            nrm_t = constp.tile([128, B_tot], f32, tag="nrm")
            nc.sync.dma_start(nrm_t[:], nrm_d[:])
            ndst_t = constp.tile([128, B_tot], f32, tag="ndst")
            nc.sync.dma_start(ndst_t[:], ndst_d[:])
            nnrm_t = constp.tile([128, B_tot], f32, tag="nnrm")
            nc.sync.dma_start(nnrm_t[:], nnrm_d[:])

            x_lo = x_d[0:SPLIT, :]
            x_hi = x_d[SPLIT:N, :]

            qc = 0  # rotate dma_gather instructions across SWDGE queues
            sc = 0  # sel build counter (for DVE/ACT split)
            for c in range(cpc):
                bl, bh = B_lo[c], B_hi[c]
                ol, oh = off_lo[c], off_hi[c]
                # dma_gather with single_packet caps at 64 descs/engine = 1024
                # idxs per instruction (HW-verified: 1280 wedges the device).
                glo_t = glop.tile([128, BLmax, C], gdt, tag="glo")
                for kb0, kb1 in _pieces(bl, 8):
                    n = (kb1 - kb0) * 128
                    nc.gpsimd.dma_gather(
                        glo_t[:, kb0:kb1, :],
                        x_lo,
                        idx_t[:, (ol + kb0) * 8 : (ol + kb1) * 8],
                        n,
                        n,
                        C,
                        queue_num=qc % N_QUEUES,
                    )
                    qc += 1
                ghi_t = ghip.tile([128, BHmax, C], gdt, tag="ghi")
                for kb0, kb1 in _pieces(bh, 8):
                    n = (kb1 - kb0) * 128
                    nc.gpsimd.dma_gather(
                        ghi_t[:, kb0:kb1, :],
                        x_hi,
                        idx_t[:, (oh + kb0) * 8 : (oh + kb1) * 8],
                        n,
                        n,
                        C,
                        queue_num=qc % N_QUEUES,
                    )
                    qc += 1

                agg_t = pagg.tile([128, CHUNK_W], mybir.dt.float32, tag="agg")
                B = bl + bh
                for b in range(B):
                    gb = (ol + b) if b < bl else (oh + b - bl)
                    sel_t = selp.tile([128, CHUNK_W], gdt, tag="sel")
                    if sc % ACT_MOD == ACT_MOD - 1:
                        # ScalarE build: t = |iota - dst|; sel = relu(nrm - nrm*t)
                        tmp_t = tmpp.tile([128, CHUNK_W], bf16, tag="tmp")
                        nc.scalar.activation(
                            tmp_t[:], iota_t[:], AB, bias=ndst_t[:, gb : gb + 1]
                        )
                        nc.scalar.activation(
                            sel_t[:],
                            tmp_t[:],
                            RL,
                            bias=nrm_t[:, gb : gb + 1],
                            scale=nnrm_t[:, gb : gb + 1],
                        )
                    else:
                        nc.vector.tensor_scalar(
                            sel_t[:],
                            iota_t[:],
                            dst_t[:, gb : gb + 1],
                            nrm_t[:, gb : gb + 1],
                            ie,
                            mu,
                        )
                    sc += 1
                    g_ap = glo_t[:, b, :] if b < bl else ghi_t[:, b - bl, :]
                    nc.tensor.matmul(
                        agg_t[:],
                        lhsT=g_ap,
                        rhs=sel_t[:],
                        start=(b == 0),
                        stop=(b == B - 1),
                    )
                aggs_t = aggsp.tile([128, CHUNK_W], f32, tag="aggs")
                nc.scalar.copy(aggs_t[:], agg_t[:])
                outp_t = pout.tile([128, CHUNK_W], mybir.dt.float32, tag="outp")
                nc.tensor.matmul(
                    outp_t[:], lhsT=w_t[:], rhs=aggs_t[:], start=True, stop=True
                )
                outs_t = outsp.tile([128, CHUNK_W], f32, tag="outs")
                nc.scalar.activation(
                    outs_t[:],
                    outp_t[:],
                    RL,
                    bias=bias_t[:, 0:1],
                    scale=1.0,
                )
                nc.sync.dma_start(out_d[:, c * CHUNK_W : (c + 1) * CHUNK_W], outs_t[:])
    nc.compile()
    return nc


def _make_in_maps(x, weight, bias, per_core, meta):
    import ml_dtypes

    gnp = ml_dtypes.bfloat16 if GATHER_DTYPE == "bf16" else np.float32
    xg = np.ascontiguousarray(np.asarray(x, dtype=np.float32).astype(gnp))
    iota = (
        np.tile(np.arange(CHUNK_W, dtype=np.float32), (128, 1)).astype(
            ml_dtypes.bfloat16
        )
    )
    w = np.ascontiguousarray(np.asarray(weight, dtype=np.float32))
    bvec = np.zeros((128, 1), np.float32)
    bvec[: len(bias), 0] = np.asarray(bias, dtype=np.float32)
    in_maps = []
    for k in range(N_CORES):
        pc = per_core[k]
        in_maps.append(
            dict(
                xg=xg,
                idx=pc["idx"],
                dstslot=pc["dstslot"],
                normv=pc["normv"],
                ndstslot=-pc["dstslot"],
                nnormv=-pc["normv"],
                iota=iota,
                weight=w,
                bias=bvec,
            )
        )
    return in_maps


def _unshard(results, meta):
    cpc = meta["cpc"]
    outs = [np.asarray(results[k]["out"], dtype=np.float32) for k in range(N_CORES)]
    big = np.concatenate(outs, axis=1)  # [128, ncores*cpc*CHUNK_W]
    chunk_of = meta["chunk_of"]
    rank = meta["rank_of_chunk"][chunk_of]
    core = chunk_of // cpc
    col = core * (cpc * CHUNK_W) + rank * CHUNK_W + meta["slot_of"]
    return np.ascontiguousarray(big[:, col].T)


def kernel(x, edge_index, weight, bias):
    x = np.asarray(x)
    per_core, meta = _host_prep(x, edge_index)
    nc = _build_program(meta)
    in_maps = _make_in_maps(x, np.asarray(weight), np.asarray(bias), per_core, meta)
    res = run_bass_kernel_spmd(
        nc,
        in_maps,
        list(range(N_CORES)),
        trace=os.environ.get("GCN_TRACE", "0") == "1",
    )
    LAST_RUN_INFO["exec_time_ns"] = res.exec_time_ns
    LAST_RUN_INFO["meta"] = {
        k: v for k, v in meta.items() if np.isscalar(v)
    } | dict(B_tot=meta["B_tot"])
    return _unshard(res.results, meta)
